# revision 30
# baseline (speedup 1.0000x reference)
"""MoE routing kernel (nn_HEA_10462540333708) for 8 Trainium2 NeuronCores.

Reference computation (B=16384, T=2, D=1024, DE=512, S=4, P=4):
    x = stack([x0, x1], 1)                                # [B, T, D]
    gates = softmax(x @ W_gate + b_gate)                  # [B, T, S+P]
    share = relu(x @ W_share + b_share)                   # [B, T, S, DE]
    spcf  = relu(x @ W_spcf  + b_spcf)                    # [B, T, P, DE]
    out   = einsum('bte,btef->btf', gates, [share|spcf])  # [B, T, DE]

Strategy: data-parallel over B across 8 cores (2048 rows each, weights
replicated, no collectives).  Per (task, m-tile) the 8 gate columns and
the 8*512 expert columns are fused into one 4104-column weight matrix
laid out as [gates(8) | e0..e7] and streamed through the PE in nine
column-chunks (<=512 so each chunk's fp32 accumulator fits one PSUM
bank).  The gate logits ride along in chunk 0's matmuls, so no tiny
N=8 matmuls (those cost ~42us of PE time in the old schedule).

Mixed precision: chunk 0 (gates + e0 head) runs bf16 on all 8 k-tiles;
chunks 1-8 contract k-tiles 0-1 with a single fp8-e4m3 matmul in
DoubleRowSwInterleave mode (~1.8x PE throughput; plain DoubleRow does
NOT double-pump on this hw) and k-tiles 2-7 in bf16.  x is scaled by 16
and W by 1024 so fp8 uses its range; the 1/16384 descale folds into the
relu and softmax activation scale.  Measured rel-err 1.922e-2 vs the
fp32 reference (deterministic), inside the 2e-2 budget.
"""

import numpy as np
import ml_dtypes

B, T, D, DE, S, P = 16384, 2, 1024, 512, 4, 4
NCORES = 8
BC = B // NCORES          # rows per core
MT = BC // 128            # m-tiles per task per core
KT = D // 128             # 128-deep contraction tiles
NE = S + P                # experts per task
TCOLS = NE * DE + NE      # 4104: [gates 8 | e0..e7 each 512]

# column chunks (task-layout offsets); each <=512 so one PSUM bank holds it
CB = [0, 512, 1024, 1536, 2048, 2560, 3072, 3416, 3760, 4104]
NCH = len(CB) - 1
DR_CHUNKS = frozenset(range(1, NCH))  # chunks using fp8 DoubleRow on k0-1
# "bf16": pure bf16 (rel-err 2.4e-3)
# "swi": fp8 e4m3 DoubleRowSwInterleave on k0-1 of chunks 1-8 (rel-err 1.9e-2)
#        plain "dr" measured no double-pumping on this hw (1 row/cycle), so swi
#        is the only fp8 mode worth running, and only if it actually pumps.
MODE = "swi"
USE_DR = MODE == "swi"

XS, WS = 16.0, 1024.0     # fp8 range scaling for x and W
HS = XS * WS

# deduped SBUF column arrangement:
#   [shared c1..c3 (1536) | t0: c0,c4..c8 (2568) | t1: c0,c4..c8 (2568)]
NB67 = 1536 + 2 * 2568    # bf16 k2-7 columns
NB8 = 1536 + 2 * 2056     # fp8 k0-1 columns (no c0)
NWARM = 2                 # PE warmup dummy matmuls

BF16 = ml_dtypes.bfloat16
F8 = ml_dtypes.float8_e4m3

_cache: dict = {}


def _map67(t, lo):
    """task-layout col lo -> wb67 sbuf col (chunks never straddle regions)."""
    if 512 <= lo < 2048:
        return lo - 512
    if lo < 512:
        return 1536 + t * 2568 + lo
    return 1536 + t * 2568 + 512 + (lo - 2048)


def _map8(t, lo):
    if 512 <= lo < 2048:
        return lo - 512
    return 1536 + t * 2056 + (lo - 2048)


def _segments(j):
    """chunk j -> list of (expert, chunk-col range, feature range)."""
    lo, hi = CB[j], CB[j + 1]
    segs = []
    for e in range(NE):
        s, t_ = NE + DE * e, NE + DE * (e + 1)
        a, b = max(lo, s), min(hi, t_)
        if a < b:
            segs.append((e, a - lo, b - lo, a - s, b - s))
    return segs


def _build_bass(mt, has_bias, use_dr):
    import concourse.bacc as bacc
    import concourse.mybir as mybir
    import concourse.tile as tile

    f32 = mybir.dt.float32
    bf16 = mybir.dt.bfloat16
    f8 = mybir.dt.float8e4
    AX = mybir.AxisListType.X
    AF = mybir.ActivationFunctionType
    ALU = mybir.AluOpType
    DR = mybir.MatmulPerfMode.DoubleRow

    nc = bacc.Bacc("TRN2", target_bir_lowering=False, debug=False)

    DRSWI = mybir.MatmulPerfMode.DoubleRowSwInterleave

    xb = nc.dram_tensor("xb", [T, mt, 128, 6, 128], bf16, kind="ExternalInput").ap()
    xb01 = nc.dram_tensor("xb01", [T, mt, 128, 2, 128], bf16, kind="ExternalInput").ap()
    # x8 holds the k0-1 slice sw-interleaved for DoubleRowSwInterleave:
    # col 2j   = x_k0[:, 127-j],  col 2j+1 = x_k1[:, 127-j]
    x8 = nc.dram_tensor("x8", [T, mt, 128, 256], f8, kind="ExternalInput").ap()
    wb67 = nc.dram_tensor("wb67", [128, 6, NB67], bf16, kind="ExternalInput").ap()
    wb01 = nc.dram_tensor("wb01", [T, 128, 2, TCOLS], bf16, kind="ExternalInput").ap()
    # chunk-0 weights, all 8 k-tiles, contiguous for a full-bandwidth head DMA
    wc0 = nc.dram_tensor("wc0", [T, 128, 8, 512], bf16, kind="ExternalInput").ap()
    w8 = nc.dram_tensor("w8", [128, 2, NB8], f8, kind="ExternalInput").ap()
    out = nc.dram_tensor("out", [T, mt, 128, DE], f32, kind="ExternalOutput").ap()
    if has_bias:
        bec = nc.dram_tensor("bec", [T, 128, TCOLS], f32, kind="ExternalInput").ap()

    NH = min(4, mt)  # head m-tiles processed chunk-major during the W load
    ORDER = [(t, m) for t in range(T) for m in range(mt)]

    with tile.TileContext(nc) as tc:
        with (
            tc.tile_pool(name="wp", bufs=1) as wpool,
            tc.tile_pool(name="xp", bufs=6) as xpool,
            tc.tile_pool(name="hps", bufs=8, space="PSUM") as hpool,
            tc.tile_pool(name="act", bufs=6) as apool,
            tc.tile_pool(name="accp", bufs=6) as accpool,
            tc.tile_pool(name="soft", bufs=5) as spool,
        ):
            # --- PE warmup: dummy matmuls on a zeroed tile, overlapping DMAs
            dmt = wpool.tile([128, 512], bf16, tag="dummy")
            nc.vector.memset(dmt[:], 0)
            for i in range(NWARM):
                wp_ = hpool.tile([128, 512], f32, tag="h")
                nc.tensor.matmul(wp_[:], lhsT=dmt[:, 0:128], rhs=dmt[:], start=True, stop=True)

            # --- x tiles: DMA per (t, m); first 6 m-tiles up front
            xtiles = {}

            def fetch_x(t, m):
                bt = xpool.tile([128, 6, 128], bf16, tag="xb")
                b01t = xpool.tile([128, 2, 128], bf16, tag="xb01")
                q8t = None
                nc.sync.dma_start(out=bt[:], in_=xb[t, m])
                nc.sync.dma_start(out=b01t[:], in_=xb01[t, m])
                if use_dr:
                    q8t = xpool.tile([128, 256], f8, tag="x8")
                    nc.sync.dma_start(out=q8t[:], in_=x8[t, m])
                xtiles[(t, m)] = (bt, b01t, q8t)

            fetch_x(*ORDER[0])  # m0's x first, then chunk-0 weights, then more x

            # --- weight tiles; DMA slices in consumption order
            wb67t = wpool.tile([128, 6, NB67], bf16, tag="wb67")
            wb01t = (
                [
                    wpool.tile([128, 2, TCOLS], bf16, tag=f"wb01_{t}", name=f"wb01t{t}")
                    for t in range(T)
                ]
                if not (use_dr and DR_CHUNKS >= frozenset(range(1, NCH)))
                else None
            )
            w8t = (
                wpool.tile([128, 2, NB8], f8, tag="w8", name="w8t") if use_dr else None
            )
            wc0t = [
                wpool.tile([128, 8, 512], bf16, tag=f"wc0_{t}", name=f"wc0t{t}")
                for t in range(T)
            ]
            bect = None
            if has_bias:
                bect = [
                    wpool.tile([128, TCOLS], f32, tag=f"bec{t}", name=f"bect{t}")
                    for t in range(T)
                ]
                for t in range(T):
                    nc.sync.dma_start(out=bect[t][:], in_=bec[t])

            def fetch_w_chunk(t, j, eng=None):
                # head-critical chunks go on the (idle) gpsimd queue so their
                # descriptor prep overlaps the x DMAs on the sync queue
                eng = eng or nc.sync
                lo, hi = CB[j], CB[j + 1]
                n = hi - lo
                if j == 0:
                    if use_dr:
                        eng.dma_start(out=wc0t[t][:], in_=wc0[t])
                        return
                    eng.dma_start(
                        out=wb01t[t][:, :, lo : lo + n], in_=wb01[t][:, :, lo : lo + n]
                    )
                elif use_dr and j in DR_CHUNKS:
                    o8 = _map8(t, lo)
                    eng.dma_start(
                        out=w8t[:, :, o8 : o8 + n], in_=w8[:, :, o8 : o8 + n]
                    )
                else:
                    eng.dma_start(
                        out=wb01t[t][:, :, lo : lo + n], in_=wb01[t][:, :, lo : lo + n]
                    )
                o67 = _map67(t, lo)
                eng.dma_start(
                    out=wb67t[:, :, o67 : o67 + n], in_=wb67[:, :, o67 : o67 + n]
                )

            # weights per chunk in consumption order; shared chunks (1-3)
            # once.  Chunk 0 + one lookahead first, then the rest of the head
            # x tiles, so the PE can start as soon as chunk 0 lands.
            fetch_w_chunk(0, 0, eng=nc.gpsimd)
            fetch_w_chunk(0, 1, eng=nc.gpsimd)
            for tm in ORDER[1:6]:
                fetch_x(*tm)
            fetch_w_chunk(0, 2, eng=nc.gpsimd)
            for j in range(3, NCH):
                fetch_w_chunk(0, j)
            for j in [0, 4, 5, 6, 7, 8]:
                fetch_w_chunk(1, j)

            # --- compute per (t, m)
            def chunk_mms(t, m, j):
                lo, hi = CB[j], CB[j + 1]
                n = hi - lo
                bt, b01t, q8t = xtiles[(t, m)]
                hp = hpool.tile([128, 512], f32, tag="h")
                if j == 0 and use_dr:
                    for k in range(8):
                        nc.tensor.matmul(
                            hp[:, 0:n],
                            lhsT=b01t[:, k, :] if k < 2 else bt[:, k - 2, :],
                            rhs=wc0t[t][:, k, :],
                            start=(k == 0),
                            stop=(k == 7),
                        )
                    return hp
                if j != 0 and use_dr and j in DR_CHUNKS:
                    o8 = _map8(t, lo)
                    nc.tensor.matmul(
                        hp[:, 0:n],
                        lhsT=q8t[:],
                        rhs=w8t[:, :, o8 : o8 + n],
                        start=True,
                        stop=False,
                        perf_mode=DRSWI,
                    )
                else:
                    # shared chunks (cols 512:2048) are identical across tasks
                    t01 = 0 if 512 <= lo < 2048 else t
                    for k in range(2):
                        nc.tensor.matmul(
                            hp[:, 0:n],
                            lhsT=b01t[:, k, :],
                            rhs=wb01t[t01][:, k, lo:hi],
                            start=(k == 0),
                            stop=False,
                        )
                o67 = _map67(t, lo)
                for k in range(6):
                    nc.tensor.matmul(
                        hp[:, 0:n],
                        lhsT=bt[:, k, :],
                        rhs=wb67t[:, k, o67 : o67 + n],
                        start=False,
                        stop=(k == 5),
                    )
                return hp

            def drain_chunk(t, m, j, hp, acc, gstate):
                lo, hi = CB[j], CB[j + 1]
                n = hi - lo
                src = hp
                if has_bias:
                    src = apool.tile([128, 512], f32, tag="ab")
                    nc.vector.tensor_tensor(
                        src[:, 0:n], hp[:, 0:n], bect[t][:, lo:hi], op=ALU.add
                    )
                if j == 0:
                    # softmax over the 8 gate logits (scaled by HS)
                    negmax = spool.tile([128, 1], f32, tag="negmax")
                    nc.vector.tensor_reduce(
                        negmax[:], src[:, 0:NE], axis=AX, op=ALU.max, negate=True
                    )
                    negmax2 = spool.tile([128, 1], f32, tag="negmax2")
                    nc.scalar.activation(negmax2[:], negmax[:], AF.Copy, scale=1.0 / HS)
                    expg = spool.tile([128, NE], f32, tag="expg")
                    nc.scalar.activation(
                        expg[:], src[:, 0:NE], AF.Exp, bias=negmax2[:], scale=1.0 / HS
                    )
                    ssum = spool.tile([128, 1], f32, tag="ssum")
                    nc.vector.tensor_reduce(ssum[:], expg[:], axis=AX, op=ALU.add)
                    rinv = spool.tile([128, 1], f32, tag="rinv")
                    nc.vector.reciprocal(rinv[:], ssum[:])
                    gsb = spool.tile([128, NE], f32, tag="gsb")
                    nc.vector.tensor_scalar_mul(gsb[:], expg[:], rinv[:])
                    gstate["gsb"] = gsb
                gsb = gstate["gsb"]
                a = apool.tile([128, 512], f32, tag="a")
                segs = _segments(j)
                ca0 = segs[0][1]  # skip gate cols in chunk 0
                nc.scalar.activation(a[:, ca0:n], src[:, ca0:n], AF.Relu, scale=1.0 / HS)
                for e, ca, cb_, fa, fb in segs:
                    if (j == 0) or (j == 1 and fa == DE - NE):
                        nc.vector.tensor_scalar_mul(
                            acc[:, fa:fb], a[:, ca:cb_], gsb[:, e : e + 1]
                        )
                    else:
                        nc.vector.scalar_tensor_tensor(
                            acc[:, fa:fb], a[:, ca:cb_], gsb[:, e : e + 1],
                            acc[:, fa:fb], op0=ALU.mult, op1=ALU.add,
                        )

            accs = {}
            gstates = {}

            def start_tile(t, m):
                accs[(t, m)] = accpool.tile([128, DE], f32, tag="acc", name="acc")
                gstates[(t, m)] = {}

            def finish_tile(t, m):
                nc.gpsimd.dma_start(out=out[t, m], in_=accs[(t, m)][:])
                del accs[(t, m)], gstates[(t, m)], xtiles[(t, m)]

            # head: chunk-major over the first NH m-tiles while weights land
            for tm in ORDER[:NH]:
                start_tile(*tm)
            for j in range(NCH):
                for t, m in ORDER[:NH]:
                    hp = chunk_mms(t, m, j)
                    drain_chunk(t, m, j, hp, accs[(t, m)], gstates[(t, m)])
            for tm in ORDER[:NH]:
                finish_tile(*tm)

            # steady state: m-major
            for i in range(NH, len(ORDER)):
                t, m = ORDER[i]
                if i + 2 < len(ORDER) and ORDER[i + 2] not in xtiles:
                    fetch_x(*ORDER[i + 2])
                start_tile(t, m)
                for j in range(NCH):
                    hp = chunk_mms(t, m, j)
                    drain_chunk(t, m, j, hp, accs[(t, m)], gstates[(t, m)])
                finish_tile(t, m)
    nc.compile()
    return nc


def _prep_w(W_share, W_spcf, W_gate):
    # per-task column layout [gates 8 | shared e0..3 | specific e4..7], scaled
    wt = []
    for t in range(T):
        cols = np.concatenate(
            [
                W_gate[t],
                np.transpose(W_share, (1, 0, 2)).reshape(D, S * DE),
                np.transpose(W_spcf[t], (1, 0, 2)).reshape(D, P * DE),
            ],
            axis=1,
        ) * WS
        wt.append(cols)  # [D, TCOLS] f32

    big67 = np.concatenate(
        [wt[0][:, 512:2048], wt[0][:, 0:512], wt[0][:, 2048:], wt[1][:, 0:512], wt[1][:, 2048:]],
        axis=1,
    )
    wb67 = np.ascontiguousarray(
        big67.reshape(KT, 128, NB67)[2:8].transpose(1, 0, 2)
    ).astype(BF16)
    wb01 = np.ascontiguousarray(
        np.stack([w[:256].reshape(2, 128, TCOLS).transpose(1, 0, 2) for w in wt], 0)
    ).astype(BF16)
    big8 = np.concatenate(
        [wt[0][:256, 512:2048], wt[0][:256, 2048:], wt[1][:256, 2048:]], axis=1
    )
    w8 = np.ascontiguousarray(big8.reshape(2, 128, NB8).transpose(1, 0, 2)).astype(F8)
    wc0 = np.ascontiguousarray(
        np.stack(
            [w[:, 0:512].reshape(KT, 128, 512).transpose(1, 0, 2) for w in wt], 0
        )
    ).astype(BF16)
    return wb67, wb01, w8, wc0


def _prep_x(x, core, mt=MT):
    # [BC, D] -> [mt, 128p(d%128), KT, 128b] scaled; split k0-1 / k2-7
    xc = x[core * BC : core * BC + mt * 128] * XS
    xt = xc.reshape(mt, 128, KT, 128).transpose(0, 3, 2, 1)
    xb = np.ascontiguousarray(xt[:, :, 2:8]).astype(BF16)
    xb01 = np.ascontiguousarray(xt[:, :, 0:2]).astype(BF16)
    # sw-interleave for DoubleRowSwInterleave: [m, p, 256] with
    # col 2j = x_k0[:, 127-j], col 2j+1 = x_k1[:, 127-j]
    x8i = np.empty((mt, 128, 256), np.float32)
    x8i[:, :, 0::2] = xt[:, :, 0, ::-1]
    x8i[:, :, 1::2] = xt[:, :, 1, ::-1]
    x8 = np.ascontiguousarray(x8i).astype(F8)
    return xb, xb01, x8


def _prep_bias(b_share, b_spcf, b_gate):
    bec = np.zeros((T, TCOLS), np.float32)
    for t in range(T):
        bec[t, 0:NE] = np.asarray(b_gate, np.float32).reshape(T, NE)[t]
        bec[t, NE : NE + S * DE] = np.asarray(b_share, np.float32).reshape(S * DE)
        bec[t, NE + S * DE :] = np.asarray(b_spcf, np.float32).reshape(T, P * DE)[t]
    bec *= HS
    return np.ascontiguousarray(np.broadcast_to(bec[:, None], (T, 128, TCOLS)))


def kernel(x0, x1, W_share, b_share, W_spcf, b_spcf, W_gate, b_gate):
    from concourse.bass_utils import run_bass_kernel_spmd

    has_bias = bool(
        np.any(np.asarray(b_share)) or np.any(np.asarray(b_spcf)) or np.any(np.asarray(b_gate))
    )
    key = (MT, has_bias, USE_DR)
    if key not in _cache:
        _cache[key] = _build_bass(*key)
    nc = _cache[key]

    wb67, wb01, w8, wc0 = _prep_w(
        np.asarray(W_share, np.float32),
        np.asarray(W_spcf, np.float32),
        np.asarray(W_gate, np.float32),
    )
    xs = [np.asarray(x0, np.float32), np.asarray(x1, np.float32)]

    in_maps = []
    for c in range(NCORES):
        parts = [_prep_x(xs[t], c) for t in range(T)]
        m = {
            "xb": np.ascontiguousarray(np.stack([p[0] for p in parts], 0)),
            "xb01": np.ascontiguousarray(np.stack([p[1] for p in parts], 0)),
            "x8": np.ascontiguousarray(np.stack([p[2] for p in parts], 0)),
            "wb67": wb67,
            "wb01": wb01,
            "w8": w8,
            "wc0": wc0,
        }
        if has_bias:
            m["bec"] = _prep_bias(b_share, b_spcf, b_gate)
        in_maps.append(m)

    res = run_bass_kernel_spmd(nc, in_maps, core_ids=list(range(NCORES)))
    global _last_results
    _last_results = res

    outp = np.empty((B, T, DE), np.float32)
    for c in range(NCORES):
        oc = res.results[c]["out"]  # [T, MT, 128, DE]
        outp[c * BC : (c + 1) * BC] = oc.transpose(1, 2, 0, 3).reshape(BC, T, DE)
    return outp


# revision 32
# speedup vs baseline: 1.1803x; 1.1803x over previous
"""MoE routing kernel (nn_HEA_10462540333708) for 8 Trainium2 NeuronCores.

Reference computation (B=16384, T=2, D=1024, DE=512, S=4, P=4):
    x = stack([x0, x1], 1)                                # [B, T, D]
    gates = softmax(x @ W_gate + b_gate)                  # [B, T, S+P]
    share = relu(x @ W_share + b_share)                   # [B, T, S, DE]
    spcf  = relu(x @ W_spcf  + b_spcf)                    # [B, T, P, DE]
    out   = einsum('bte,btef->btf', gates, [share|spcf])  # [B, T, DE]

Strategy: data-parallel over B across 8 cores (2048 rows each, weights
replicated, no collectives).  Per (task, m-tile) the 8 gate columns and
the 8*512 expert columns are fused into one 4104-column weight matrix
laid out as [gates(8) | e0..e7] and streamed through the PE in nine
column-chunks (<=512 so each chunk's fp32 accumulator fits one PSUM
bank).  The gate logits ride along in chunk 0's matmuls, so no tiny
N=8 matmuls (those cost ~42us of PE time in the old schedule).

Mixed precision: chunk 0 (gates + e0 head) runs bf16 on all 8 k-tiles;
chunks 1-8 contract k-tiles 0-1 with a single fp8-e4m3 matmul in
DoubleRowSwInterleave mode (~1.8x PE throughput; plain DoubleRow does
NOT double-pump on this hw) and k-tiles 2-7 in bf16.  x is scaled by 16
and W by 1024 so fp8 uses its range; the 1/16384 descale folds into the
relu and softmax activation scale.  Measured rel-err 1.922e-2 vs the
fp32 reference (deterministic), inside the 2e-2 budget.
"""

import numpy as np
import ml_dtypes

B, T, D, DE, S, P = 16384, 2, 1024, 512, 4, 4
NCORES = 8
BC = B // NCORES          # rows per core
MT = BC // 128            # m-tiles per task per core
KT = D // 128             # 128-deep contraction tiles
NE = S + P                # experts per task
TCOLS = NE * DE + NE      # 4104: [gates 8 | e0..e7 each 512]

# column chunks (task-layout offsets); each <=512 so one PSUM bank holds it
CB = [0, 512, 1024, 1536, 2048, 2560, 3072, 3416, 3760, 4104]
NCH = len(CB) - 1
DR_CHUNKS = frozenset(range(1, NCH))  # chunks using fp8 DoubleRow on k0-1
# "bf16": pure bf16 (rel-err 2.4e-3)
# "swi": fp8 e4m3 DoubleRowSwInterleave on k0-1 of chunks 1-8 (rel-err 1.9e-2)
#        plain "dr" measured no double-pumping on this hw (1 row/cycle), so swi
#        is the only fp8 mode worth running, and only if it actually pumps.
MODE = "swi"
USE_DR = MODE == "swi"

XS, WS = 16.0, 1024.0     # fp8 range scaling for x and W
HS = XS * WS

# deduped SBUF column arrangement:
#   [shared c1..c3 (1536) | t0: c0,c4..c8 (2568) | t1: c0,c4..c8 (2568)]
NB67 = 1536 + 2 * 2568    # bf16 k2-7 columns
NB8 = 1536 + 2 * 2056     # fp8 k0-1 columns (no c0)
NWARM = 2                 # PE warmup dummy matmuls

BF16 = ml_dtypes.bfloat16
F8 = ml_dtypes.float8_e4m3

_cache: dict = {}


def _map67(t, lo):
    """task-layout col lo -> wb67 sbuf col (chunks never straddle regions)."""
    if 512 <= lo < 2048:
        return lo - 512
    if lo < 512:
        return 1536 + t * 2568 + lo
    return 1536 + t * 2568 + 512 + (lo - 2048)


def _map8(t, lo):
    if 512 <= lo < 2048:
        return lo - 512
    return 1536 + t * 2056 + (lo - 2048)


def _segments(j):
    """chunk j -> list of (expert, chunk-col range, feature range)."""
    lo, hi = CB[j], CB[j + 1]
    segs = []
    for e in range(NE):
        s, t_ = NE + DE * e, NE + DE * (e + 1)
        a, b = max(lo, s), min(hi, t_)
        if a < b:
            segs.append((e, a - lo, b - lo, a - s, b - s))
    return segs


def _build_bass(mt, has_bias, use_dr):
    import concourse.bacc as bacc
    import concourse.mybir as mybir
    import concourse.tile as tile

    f32 = mybir.dt.float32
    bf16 = mybir.dt.bfloat16
    f8 = mybir.dt.float8e4
    AX = mybir.AxisListType.X
    AF = mybir.ActivationFunctionType
    ALU = mybir.AluOpType
    DR = mybir.MatmulPerfMode.DoubleRow

    nc = bacc.Bacc("TRN2", target_bir_lowering=False, debug=False)

    DRSWI = mybir.MatmulPerfMode.DoubleRowSwInterleave

    xb = nc.dram_tensor("xb", [T, mt, 128, 6, 128], bf16, kind="ExternalInput").ap()
    xb01 = nc.dram_tensor("xb01", [T, mt, 128, 2, 128], bf16, kind="ExternalInput").ap()
    # x8 holds the k0-1 slice sw-interleaved for DoubleRowSwInterleave:
    # col 2j   = x_k0[:, 127-j],  col 2j+1 = x_k1[:, 127-j]
    x8 = nc.dram_tensor("x8", [T, mt, 128, 256], f8, kind="ExternalInput").ap()
    wb67 = nc.dram_tensor("wb67", [128, 6, NB67], bf16, kind="ExternalInput").ap()
    wb01 = nc.dram_tensor("wb01", [T, 128, 2, TCOLS], bf16, kind="ExternalInput").ap()
    # chunk-0 weights, all 8 k-tiles, contiguous for a full-bandwidth head DMA
    wc0 = nc.dram_tensor("wc0", [T, 128, 8, 512], bf16, kind="ExternalInput").ap()
    w8 = nc.dram_tensor("w8", [128, 2, NB8], f8, kind="ExternalInput").ap()
    out = nc.dram_tensor("out", [T, mt, 128, DE], f32, kind="ExternalOutput").ap()
    if has_bias:
        bec = nc.dram_tensor("bec", [T, 128, TCOLS], f32, kind="ExternalInput").ap()

    NH = min(4, mt)  # head m-tiles processed chunk-major during the W load
    ORDER = [(t, m) for t in range(T) for m in range(mt)]

    with tile.TileContext(nc) as tc:
        with (
            tc.tile_pool(name="wp", bufs=1) as wpool,
            tc.tile_pool(name="xp", bufs=6) as xpool,
            tc.tile_pool(name="hps", bufs=8, space="PSUM") as hpool,
            tc.tile_pool(name="act", bufs=6) as apool,
            tc.tile_pool(name="accp", bufs=6) as accpool,
            tc.tile_pool(name="soft", bufs=5) as spool,
        ):
            # --- PE warmup: dummy matmuls on a zeroed tile, overlapping DMAs
            dmt = wpool.tile([128, 512], bf16, tag="dummy")
            nc.vector.memset(dmt[:], 0)
            for i in range(NWARM):
                wp_ = hpool.tile([128, 512], f32, tag="h")
                nc.tensor.matmul(wp_[:], lhsT=dmt[:, 0:128], rhs=dmt[:], start=True, stop=True)

            # --- x tiles: DMA per (t, m); first 6 m-tiles up front
            xtiles = {}

            def fetch_x(t, m):
                bt = xpool.tile([128, 6, 128], bf16, tag="xb")
                b01t = xpool.tile([128, 2, 128], bf16, tag="xb01")
                q8t = None
                nc.sync.dma_start(out=bt[:], in_=xb[t, m])
                nc.sync.dma_start(out=b01t[:], in_=xb01[t, m])
                if use_dr:
                    q8t = xpool.tile([128, 256], f8, tag="x8")
                    nc.sync.dma_start(out=q8t[:], in_=x8[t, m])
                xtiles[(t, m)] = (bt, b01t, q8t)

            fetch_x(*ORDER[0])  # m0's x first, then chunk-0 weights, then more x

            # --- weight tiles; DMA slices in consumption order
            wb67t = wpool.tile([128, 6, NB67], bf16, tag="wb67")
            wb01t = (
                [
                    wpool.tile([128, 2, TCOLS], bf16, tag=f"wb01_{t}", name=f"wb01t{t}")
                    for t in range(T)
                ]
                if not (use_dr and DR_CHUNKS >= frozenset(range(1, NCH)))
                else None
            )
            w8t = (
                wpool.tile([128, 2, NB8], f8, tag="w8", name="w8t") if use_dr else None
            )
            wc0t = [
                wpool.tile([128, 8, 512], bf16, tag=f"wc0_{t}", name=f"wc0t{t}")
                for t in range(T)
            ]
            bect = None
            if has_bias:
                bect = [
                    wpool.tile([128, TCOLS], f32, tag=f"bec{t}", name=f"bect{t}")
                    for t in range(T)
                ]
                for t in range(T):
                    nc.sync.dma_start(out=bect[t][:], in_=bec[t])

            def fetch_w_chunk(t, j, eng=None):
                # head-critical chunks go on the (idle) gpsimd queue so their
                # descriptor prep overlaps the x DMAs on the sync queue
                eng = eng or nc.sync
                lo, hi = CB[j], CB[j + 1]
                n = hi - lo
                if j == 0:
                    if use_dr:
                        # per-k slices: the first matmul only waits for k0's
                        # 128KB instead of the whole 1MB chunk-0 tensor
                        for k in range(KT):
                            eng.dma_start(out=wc0t[t][:, k, :], in_=wc0[t][:, k, :])
                        return
                    eng.dma_start(
                        out=wb01t[t][:, :, lo : lo + n], in_=wb01[t][:, :, lo : lo + n]
                    )
                elif use_dr and j in DR_CHUNKS:
                    o8 = _map8(t, lo)
                    eng.dma_start(
                        out=w8t[:, :, o8 : o8 + n], in_=w8[:, :, o8 : o8 + n]
                    )
                else:
                    eng.dma_start(
                        out=wb01t[t][:, :, lo : lo + n], in_=wb01[t][:, :, lo : lo + n]
                    )
                o67 = _map67(t, lo)
                eng.dma_start(
                    out=wb67t[:, :, o67 : o67 + n], in_=wb67[:, :, o67 : o67 + n]
                )

            # weights per chunk in consumption order; shared chunks (1-3)
            # once.  Chunk 0 + one lookahead first, then the rest of the head
            # x tiles, so the PE can start as soon as chunk 0 lands.
            fetch_w_chunk(0, 0)
            fetch_w_chunk(0, 1)
            for tm in ORDER[1:6]:
                fetch_x(*tm)
            for j in range(2, NCH):
                fetch_w_chunk(0, j)
            for j in [0, 4, 5, 6, 7, 8]:
                fetch_w_chunk(1, j)

            # --- compute per (t, m)
            def chunk_mms(t, m, j):
                lo, hi = CB[j], CB[j + 1]
                n = hi - lo
                bt, b01t, q8t = xtiles[(t, m)]
                hp = hpool.tile([128, 512], f32, tag="h")
                if j == 0 and use_dr:
                    for k in range(8):
                        nc.tensor.matmul(
                            hp[:, 0:n],
                            lhsT=b01t[:, k, :] if k < 2 else bt[:, k - 2, :],
                            rhs=wc0t[t][:, k, :],
                            start=(k == 0),
                            stop=(k == 7),
                        )
                    return hp
                if j != 0 and use_dr and j in DR_CHUNKS:
                    o8 = _map8(t, lo)
                    nc.tensor.matmul(
                        hp[:, 0:n],
                        lhsT=q8t[:],
                        rhs=w8t[:, :, o8 : o8 + n],
                        start=True,
                        stop=False,
                        perf_mode=DRSWI,
                    )
                else:
                    # shared chunks (cols 512:2048) are identical across tasks
                    t01 = 0 if 512 <= lo < 2048 else t
                    for k in range(2):
                        nc.tensor.matmul(
                            hp[:, 0:n],
                            lhsT=b01t[:, k, :],
                            rhs=wb01t[t01][:, k, lo:hi],
                            start=(k == 0),
                            stop=False,
                        )
                o67 = _map67(t, lo)
                for k in range(6):
                    nc.tensor.matmul(
                        hp[:, 0:n],
                        lhsT=bt[:, k, :],
                        rhs=wb67t[:, k, o67 : o67 + n],
                        start=False,
                        stop=(k == 5),
                    )
                return hp

            def drain_chunk(t, m, j, hp, acc, gstate):
                lo, hi = CB[j], CB[j + 1]
                n = hi - lo
                src = hp
                if has_bias:
                    src = apool.tile([128, 512], f32, tag="ab")
                    nc.vector.tensor_tensor(
                        src[:, 0:n], hp[:, 0:n], bect[t][:, lo:hi], op=ALU.add
                    )
                if j == 0:
                    # softmax over the 8 gate logits (scaled by HS)
                    negmax = spool.tile([128, 1], f32, tag="negmax")
                    nc.vector.tensor_reduce(
                        negmax[:], src[:, 0:NE], axis=AX, op=ALU.max, negate=True
                    )
                    negmax2 = spool.tile([128, 1], f32, tag="negmax2")
                    nc.scalar.activation(negmax2[:], negmax[:], AF.Copy, scale=1.0 / HS)
                    expg = spool.tile([128, NE], f32, tag="expg")
                    nc.scalar.activation(
                        expg[:], src[:, 0:NE], AF.Exp, bias=negmax2[:], scale=1.0 / HS
                    )
                    ssum = spool.tile([128, 1], f32, tag="ssum")
                    nc.vector.tensor_reduce(ssum[:], expg[:], axis=AX, op=ALU.add)
                    rinv = spool.tile([128, 1], f32, tag="rinv")
                    nc.vector.reciprocal(rinv[:], ssum[:])
                    gsb = spool.tile([128, NE], f32, tag="gsb")
                    nc.vector.tensor_scalar_mul(gsb[:], expg[:], rinv[:])
                    gstate["gsb"] = gsb
                gsb = gstate["gsb"]
                a = apool.tile([128, 512], f32, tag="a")
                segs = _segments(j)
                ca0 = segs[0][1]  # skip gate cols in chunk 0
                nc.scalar.activation(a[:, ca0:n], src[:, ca0:n], AF.Relu, scale=1.0 / HS)
                for e, ca, cb_, fa, fb in segs:
                    if (j == 0) or (j == 1 and fa == DE - NE):
                        nc.vector.tensor_scalar_mul(
                            acc[:, fa:fb], a[:, ca:cb_], gsb[:, e : e + 1]
                        )
                    else:
                        nc.vector.scalar_tensor_tensor(
                            acc[:, fa:fb], a[:, ca:cb_], gsb[:, e : e + 1],
                            acc[:, fa:fb], op0=ALU.mult, op1=ALU.add,
                        )

            accs = {}
            gstates = {}

            def start_tile(t, m):
                accs[(t, m)] = accpool.tile([128, DE], f32, tag="acc", name="acc")
                gstates[(t, m)] = {}

            def finish_tile(t, m):
                nc.gpsimd.dma_start(out=out[t, m], in_=accs[(t, m)][:])
                del accs[(t, m)], gstates[(t, m)], xtiles[(t, m)]

            # head: chunk-major over the first NH m-tiles while weights land
            for tm in ORDER[:NH]:
                start_tile(*tm)
            for j in range(NCH):
                for t, m in ORDER[:NH]:
                    hp = chunk_mms(t, m, j)
                    drain_chunk(t, m, j, hp, accs[(t, m)], gstates[(t, m)])
            for tm in ORDER[:NH]:
                finish_tile(*tm)

            # steady state: m-major
            for i in range(NH, len(ORDER)):
                t, m = ORDER[i]
                if i + 2 < len(ORDER) and ORDER[i + 2] not in xtiles:
                    fetch_x(*ORDER[i + 2])
                start_tile(t, m)
                for j in range(NCH):
                    hp = chunk_mms(t, m, j)
                    drain_chunk(t, m, j, hp, accs[(t, m)], gstates[(t, m)])
                finish_tile(t, m)
    nc.compile()
    return nc


def _prep_w(W_share, W_spcf, W_gate):
    # per-task column layout [gates 8 | shared e0..3 | specific e4..7], scaled
    wt = []
    for t in range(T):
        cols = np.concatenate(
            [
                W_gate[t],
                np.transpose(W_share, (1, 0, 2)).reshape(D, S * DE),
                np.transpose(W_spcf[t], (1, 0, 2)).reshape(D, P * DE),
            ],
            axis=1,
        ) * WS
        wt.append(cols)  # [D, TCOLS] f32

    big67 = np.concatenate(
        [wt[0][:, 512:2048], wt[0][:, 0:512], wt[0][:, 2048:], wt[1][:, 0:512], wt[1][:, 2048:]],
        axis=1,
    )
    wb67 = np.ascontiguousarray(
        big67.reshape(KT, 128, NB67)[2:8].transpose(1, 0, 2)
    ).astype(BF16)
    wb01 = np.ascontiguousarray(
        np.stack([w[:256].reshape(2, 128, TCOLS).transpose(1, 0, 2) for w in wt], 0)
    ).astype(BF16)
    big8 = np.concatenate(
        [wt[0][:256, 512:2048], wt[0][:256, 2048:], wt[1][:256, 2048:]], axis=1
    )
    w8 = np.ascontiguousarray(big8.reshape(2, 128, NB8).transpose(1, 0, 2)).astype(F8)
    wc0 = np.ascontiguousarray(
        np.stack(
            [w[:, 0:512].reshape(KT, 128, 512).transpose(1, 0, 2) for w in wt], 0
        )
    ).astype(BF16)
    return wb67, wb01, w8, wc0


def _prep_x(x, core, mt=MT):
    # [BC, D] -> [mt, 128p(d%128), KT, 128b] scaled; split k0-1 / k2-7
    xc = x[core * BC : core * BC + mt * 128] * XS
    xt = xc.reshape(mt, 128, KT, 128).transpose(0, 3, 2, 1)
    xb = np.ascontiguousarray(xt[:, :, 2:8]).astype(BF16)
    xb01 = np.ascontiguousarray(xt[:, :, 0:2]).astype(BF16)
    # sw-interleave for DoubleRowSwInterleave: [m, p, 256] with
    # col 2j = x_k0[:, 127-j], col 2j+1 = x_k1[:, 127-j]
    x8i = np.empty((mt, 128, 256), np.float32)
    x8i[:, :, 0::2] = xt[:, :, 0, ::-1]
    x8i[:, :, 1::2] = xt[:, :, 1, ::-1]
    x8 = np.ascontiguousarray(x8i).astype(F8)
    return xb, xb01, x8


def _prep_bias(b_share, b_spcf, b_gate):
    bec = np.zeros((T, TCOLS), np.float32)
    for t in range(T):
        bec[t, 0:NE] = np.asarray(b_gate, np.float32).reshape(T, NE)[t]
        bec[t, NE : NE + S * DE] = np.asarray(b_share, np.float32).reshape(S * DE)
        bec[t, NE + S * DE :] = np.asarray(b_spcf, np.float32).reshape(T, P * DE)[t]
    bec *= HS
    return np.ascontiguousarray(np.broadcast_to(bec[:, None], (T, 128, TCOLS)))


def kernel(x0, x1, W_share, b_share, W_spcf, b_spcf, W_gate, b_gate):
    from concourse.bass_utils import run_bass_kernel_spmd

    has_bias = bool(
        np.any(np.asarray(b_share)) or np.any(np.asarray(b_spcf)) or np.any(np.asarray(b_gate))
    )
    key = (MT, has_bias, USE_DR)
    if key not in _cache:
        _cache[key] = _build_bass(*key)
    nc = _cache[key]

    wb67, wb01, w8, wc0 = _prep_w(
        np.asarray(W_share, np.float32),
        np.asarray(W_spcf, np.float32),
        np.asarray(W_gate, np.float32),
    )
    xs = [np.asarray(x0, np.float32), np.asarray(x1, np.float32)]

    in_maps = []
    for c in range(NCORES):
        parts = [_prep_x(xs[t], c) for t in range(T)]
        m = {
            "xb": np.ascontiguousarray(np.stack([p[0] for p in parts], 0)),
            "xb01": np.ascontiguousarray(np.stack([p[1] for p in parts], 0)),
            "x8": np.ascontiguousarray(np.stack([p[2] for p in parts], 0)),
            "wb67": wb67,
            "wb01": wb01,
            "w8": w8,
            "wc0": wc0,
        }
        if has_bias:
            m["bec"] = _prep_bias(b_share, b_spcf, b_gate)
        in_maps.append(m)

    res = run_bass_kernel_spmd(nc, in_maps, core_ids=list(range(NCORES)))
    global _last_results
    _last_results = res

    outp = np.empty((B, T, DE), np.float32)
    for c in range(NCORES):
        oc = res.results[c]["out"]  # [T, MT, 128, DE]
        outp[c * BC : (c + 1) * BC] = oc.transpose(1, 2, 0, 3).reshape(BC, T, DE)
    return outp


# revision 33
# speedup vs baseline: 1.1994x; 1.0162x over previous
"""MoE routing kernel (nn_HEA_10462540333708) for 8 Trainium2 NeuronCores.

Reference computation (B=16384, T=2, D=1024, DE=512, S=4, P=4):
    x = stack([x0, x1], 1)                                # [B, T, D]
    gates = softmax(x @ W_gate + b_gate)                  # [B, T, S+P]
    share = relu(x @ W_share + b_share)                   # [B, T, S, DE]
    spcf  = relu(x @ W_spcf  + b_spcf)                    # [B, T, P, DE]
    out   = einsum('bte,btef->btf', gates, [share|spcf])  # [B, T, DE]

Strategy: data-parallel over B across 8 cores (2048 rows each, weights
replicated, no collectives).  Per (task, m-tile) the 8 gate columns and
the 8*512 expert columns are fused into one 4104-column weight matrix
laid out as [gates(8) | e0..e7] and streamed through the PE in nine
column-chunks (<=512 so each chunk's fp32 accumulator fits one PSUM
bank).  The gate logits ride along in chunk 0's matmuls, so no tiny
N=8 matmuls (those cost ~42us of PE time in the old schedule).

Mixed precision: chunk 0 (gates + e0 head) runs bf16 on all 8 k-tiles;
chunks 1-8 contract k-tiles 0-1 with a single fp8-e4m3 matmul in
DoubleRowSwInterleave mode (~1.8x PE throughput; plain DoubleRow does
NOT double-pump on this hw) and k-tiles 2-7 in bf16.  x is scaled by 16
and W by 1024 so fp8 uses its range; the 1/16384 descale folds into the
relu and softmax activation scale.  Measured rel-err 1.922e-2 vs the
fp32 reference (deterministic), inside the 2e-2 budget.
"""

import numpy as np
import ml_dtypes

B, T, D, DE, S, P = 16384, 2, 1024, 512, 4, 4
NCORES = 8
BC = B // NCORES          # rows per core
MT = BC // 128            # m-tiles per task per core
KT = D // 128             # 128-deep contraction tiles
NE = S + P                # experts per task
TCOLS = NE * DE + NE      # 4104: [gates 8 | e0..e7 each 512]

# column chunks (task-layout offsets); each <=512 so one PSUM bank holds it
CB = [0, 512, 1024, 1536, 2048, 2560, 3072, 3416, 3760, 4104]
NCH = len(CB) - 1
DR_CHUNKS = frozenset(range(1, NCH))  # chunks using fp8 DoubleRow on k0-1
# "bf16": pure bf16 (rel-err 2.4e-3)
# "swi": fp8 e4m3 DoubleRowSwInterleave on k0-1 of chunks 1-8 (rel-err 1.9e-2)
#        plain "dr" measured no double-pumping on this hw (1 row/cycle), so swi
#        is the only fp8 mode worth running, and only if it actually pumps.
MODE = "swi"
USE_DR = MODE == "swi"

XS, WS = 16.0, 1024.0     # fp8 range scaling for x and W
HS = XS * WS

# deduped SBUF column arrangement:
#   [shared c1..c3 (1536) | t0: c0,c4..c8 (2568) | t1: c0,c4..c8 (2568)]
NB67 = 1536 + 2 * 2568    # bf16 k2-7 columns
NB8 = 1536 + 2 * 2056     # fp8 k0-1 columns (no c0)
NWARM = 2                 # PE warmup dummy matmuls

BF16 = ml_dtypes.bfloat16
F8 = ml_dtypes.float8_e4m3

_cache: dict = {}


def _map67(t, lo):
    """task-layout col lo -> wb67 sbuf col (chunks never straddle regions)."""
    if 512 <= lo < 2048:
        return lo - 512
    if lo < 512:
        return 1536 + t * 2568 + lo
    return 1536 + t * 2568 + 512 + (lo - 2048)


def _map8(t, lo):
    if 512 <= lo < 2048:
        return lo - 512
    return 1536 + t * 2056 + (lo - 2048)


def _segments(j):
    """chunk j -> list of (expert, chunk-col range, feature range)."""
    lo, hi = CB[j], CB[j + 1]
    segs = []
    for e in range(NE):
        s, t_ = NE + DE * e, NE + DE * (e + 1)
        a, b = max(lo, s), min(hi, t_)
        if a < b:
            segs.append((e, a - lo, b - lo, a - s, b - s))
    return segs


def _build_bass(mt, has_bias, use_dr):
    import concourse.bacc as bacc
    import concourse.mybir as mybir
    import concourse.tile as tile

    f32 = mybir.dt.float32
    bf16 = mybir.dt.bfloat16
    f8 = mybir.dt.float8e4
    AX = mybir.AxisListType.X
    AF = mybir.ActivationFunctionType
    ALU = mybir.AluOpType
    DR = mybir.MatmulPerfMode.DoubleRow

    nc = bacc.Bacc("TRN2", target_bir_lowering=False, debug=False)

    DRSWI = mybir.MatmulPerfMode.DoubleRowSwInterleave

    xb = nc.dram_tensor("xb", [T, mt, 128, 6, 128], bf16, kind="ExternalInput").ap()
    xb01 = nc.dram_tensor("xb01", [T, mt, 128, 2, 128], bf16, kind="ExternalInput").ap()
    # x8 holds the k0-1 slice sw-interleaved for DoubleRowSwInterleave:
    # col 2j   = x_k0[:, 127-j],  col 2j+1 = x_k1[:, 127-j]
    x8 = nc.dram_tensor("x8", [T, mt, 128, 256], f8, kind="ExternalInput").ap()
    wb67 = nc.dram_tensor("wb67", [128, 6, NB67], bf16, kind="ExternalInput").ap()
    wb01 = nc.dram_tensor("wb01", [T, 128, 2, TCOLS], bf16, kind="ExternalInput").ap()
    # chunk-0 weights, all 8 k-tiles, contiguous for a full-bandwidth head DMA
    wc0 = nc.dram_tensor("wc0", [T, 128, 8, 512], bf16, kind="ExternalInput").ap()
    w8 = nc.dram_tensor("w8", [128, 2, NB8], f8, kind="ExternalInput").ap()
    out = nc.dram_tensor("out", [T, mt, 128, DE], f32, kind="ExternalOutput").ap()
    if has_bias:
        bec = nc.dram_tensor("bec", [T, 128, TCOLS], f32, kind="ExternalInput").ap()

    NH = min(4, mt)  # head m-tiles processed chunk-major during the W load
    ORDER = [(t, m) for t in range(T) for m in range(mt)]

    with tile.TileContext(nc) as tc:
        with (
            tc.tile_pool(name="wp", bufs=1) as wpool,
            tc.tile_pool(name="xp", bufs=6) as xpool,
            tc.tile_pool(name="hps", bufs=8, space="PSUM") as hpool,
            tc.tile_pool(name="act", bufs=6) as apool,
            tc.tile_pool(name="accp", bufs=6) as accpool,
            tc.tile_pool(name="soft", bufs=5) as spool,
        ):
            # --- PE warmup: dummy matmuls on a zeroed tile, overlapping DMAs
            dmt = wpool.tile([128, 512], bf16, tag="dummy")
            nc.vector.memset(dmt[:], 0)
            for i in range(NWARM):
                wp_ = hpool.tile([128, 512], f32, tag="h")
                nc.tensor.matmul(wp_[:], lhsT=dmt[:, 0:128], rhs=dmt[:], start=True, stop=True)

            # --- x tiles: DMA per (t, m); first 6 m-tiles up front
            xtiles = {}

            def fetch_x(t, m):
                bt = xpool.tile([128, 6, 128], bf16, tag="xb")
                b01t = xpool.tile([128, 2, 128], bf16, tag="xb01")
                q8t = None
                nc.sync.dma_start(out=bt[:], in_=xb[t, m])
                nc.sync.dma_start(out=b01t[:], in_=xb01[t, m])
                if use_dr:
                    q8t = xpool.tile([128, 256], f8, tag="x8")
                    nc.sync.dma_start(out=q8t[:], in_=x8[t, m])
                xtiles[(t, m)] = (bt, b01t, q8t)

            fetch_x(*ORDER[0])  # m0's x first, then chunk-0 weights, then more x

            # --- weight tiles; DMA slices in consumption order
            wb67t = wpool.tile([128, 6, NB67], bf16, tag="wb67")
            wb01t = (
                [
                    wpool.tile([128, 2, TCOLS], bf16, tag=f"wb01_{t}", name=f"wb01t{t}")
                    for t in range(T)
                ]
                if not (use_dr and DR_CHUNKS >= frozenset(range(1, NCH)))
                else None
            )
            w8t = (
                wpool.tile([128, 2, NB8], f8, tag="w8", name="w8t") if use_dr else None
            )
            wc0t = [
                wpool.tile([128, 8, 512], bf16, tag=f"wc0_{t}", name=f"wc0t{t}")
                for t in range(T)
            ]
            bect = None
            if has_bias:
                bect = [
                    wpool.tile([128, TCOLS], f32, tag=f"bec{t}", name=f"bect{t}")
                    for t in range(T)
                ]
                for t in range(T):
                    nc.sync.dma_start(out=bect[t][:], in_=bec[t])

            def fetch_w_chunk(t, j, eng=None):
                # head-critical chunks go on the (idle) gpsimd queue so their
                # descriptor prep overlaps the x DMAs on the sync queue
                eng = eng or nc.sync
                lo, hi = CB[j], CB[j + 1]
                n = hi - lo
                if j == 0:
                    if use_dr:
                        # one contiguous 1MB transfer; per-k slices measured
                        # worse (1KB strided lines starve the first m-tiles)
                        eng.dma_start(out=wc0t[t][:], in_=wc0[t])
                        return
                    eng.dma_start(
                        out=wb01t[t][:, :, lo : lo + n], in_=wb01[t][:, :, lo : lo + n]
                    )
                elif use_dr and j in DR_CHUNKS:
                    o8 = _map8(t, lo)
                    eng.dma_start(
                        out=w8t[:, :, o8 : o8 + n], in_=w8[:, :, o8 : o8 + n]
                    )
                else:
                    eng.dma_start(
                        out=wb01t[t][:, :, lo : lo + n], in_=wb01[t][:, :, lo : lo + n]
                    )
                o67 = _map67(t, lo)
                eng.dma_start(
                    out=wb67t[:, :, o67 : o67 + n], in_=wb67[:, :, o67 : o67 + n]
                )

            # weights per chunk in consumption order; shared chunks (1-3)
            # once.  Chunk 0 + one lookahead first, then the rest of the head
            # x tiles, so the PE can start as soon as chunk 0 lands.
            fetch_w_chunk(0, 0)
            fetch_w_chunk(0, 1)
            for tm in ORDER[1:6]:
                fetch_x(*tm)
            for j in range(2, NCH):
                fetch_w_chunk(0, j)
            for j in [0, 4, 5, 6, 7, 8]:
                fetch_w_chunk(1, j)

            # --- compute per (t, m)
            def chunk_mms(t, m, j):
                lo, hi = CB[j], CB[j + 1]
                n = hi - lo
                bt, b01t, q8t = xtiles[(t, m)]
                hp = hpool.tile([128, 512], f32, tag="h")
                if j == 0 and use_dr:
                    for k in range(8):
                        nc.tensor.matmul(
                            hp[:, 0:n],
                            lhsT=b01t[:, k, :] if k < 2 else bt[:, k - 2, :],
                            rhs=wc0t[t][:, k, :],
                            start=(k == 0),
                            stop=(k == 7),
                        )
                    return hp
                if j != 0 and use_dr and j in DR_CHUNKS:
                    o8 = _map8(t, lo)
                    nc.tensor.matmul(
                        hp[:, 0:n],
                        lhsT=q8t[:],
                        rhs=w8t[:, :, o8 : o8 + n],
                        start=True,
                        stop=False,
                        perf_mode=DRSWI,
                    )
                else:
                    # shared chunks (cols 512:2048) are identical across tasks
                    t01 = 0 if 512 <= lo < 2048 else t
                    for k in range(2):
                        nc.tensor.matmul(
                            hp[:, 0:n],
                            lhsT=b01t[:, k, :],
                            rhs=wb01t[t01][:, k, lo:hi],
                            start=(k == 0),
                            stop=False,
                        )
                o67 = _map67(t, lo)
                for k in range(6):
                    nc.tensor.matmul(
                        hp[:, 0:n],
                        lhsT=bt[:, k, :],
                        rhs=wb67t[:, k, o67 : o67 + n],
                        start=False,
                        stop=(k == 5),
                    )
                return hp

            def drain_chunk(t, m, j, hp, acc, gstate):
                lo, hi = CB[j], CB[j + 1]
                n = hi - lo
                src = hp
                if has_bias:
                    src = apool.tile([128, 512], f32, tag="ab")
                    nc.vector.tensor_tensor(
                        src[:, 0:n], hp[:, 0:n], bect[t][:, lo:hi], op=ALU.add
                    )
                if j == 0:
                    # softmax over the 8 gate logits (scaled by HS)
                    negmax = spool.tile([128, 1], f32, tag="negmax")
                    nc.vector.tensor_reduce(
                        negmax[:], src[:, 0:NE], axis=AX, op=ALU.max, negate=True
                    )
                    negmax2 = spool.tile([128, 1], f32, tag="negmax2")
                    nc.scalar.activation(negmax2[:], negmax[:], AF.Copy, scale=1.0 / HS)
                    expg = spool.tile([128, NE], f32, tag="expg")
                    nc.scalar.activation(
                        expg[:], src[:, 0:NE], AF.Exp, bias=negmax2[:], scale=1.0 / HS
                    )
                    ssum = spool.tile([128, 1], f32, tag="ssum")
                    nc.vector.tensor_reduce(ssum[:], expg[:], axis=AX, op=ALU.add)
                    rinv = spool.tile([128, 1], f32, tag="rinv")
                    nc.vector.reciprocal(rinv[:], ssum[:])
                    gsb = spool.tile([128, NE], f32, tag="gsb")
                    nc.vector.tensor_scalar_mul(gsb[:], expg[:], rinv[:])
                    gstate["gsb"] = gsb
                gsb = gstate["gsb"]
                a = apool.tile([128, 512], f32, tag="a")
                segs = _segments(j)
                ca0 = segs[0][1]  # skip gate cols in chunk 0
                nc.scalar.activation(a[:, ca0:n], src[:, ca0:n], AF.Relu, scale=1.0 / HS)
                for e, ca, cb_, fa, fb in segs:
                    if (j == 0) or (j == 1 and fa == DE - NE):
                        nc.vector.tensor_scalar_mul(
                            acc[:, fa:fb], a[:, ca:cb_], gsb[:, e : e + 1]
                        )
                    else:
                        nc.vector.scalar_tensor_tensor(
                            acc[:, fa:fb], a[:, ca:cb_], gsb[:, e : e + 1],
                            acc[:, fa:fb], op0=ALU.mult, op1=ALU.add,
                        )

            accs = {}
            gstates = {}

            def start_tile(t, m):
                accs[(t, m)] = accpool.tile([128, DE], f32, tag="acc", name="acc")
                gstates[(t, m)] = {}

            def finish_tile(t, m):
                nc.gpsimd.dma_start(out=out[t, m], in_=accs[(t, m)][:])
                del accs[(t, m)], gstates[(t, m)], xtiles[(t, m)]

            # head: chunk-major over the first NH m-tiles while weights land
            for tm in ORDER[:NH]:
                start_tile(*tm)
            for j in range(NCH):
                for t, m in ORDER[:NH]:
                    hp = chunk_mms(t, m, j)
                    drain_chunk(t, m, j, hp, accs[(t, m)], gstates[(t, m)])
            for tm in ORDER[:NH]:
                finish_tile(*tm)

            # steady state: m-major
            for i in range(NH, len(ORDER)):
                t, m = ORDER[i]
                if i + 2 < len(ORDER) and ORDER[i + 2] not in xtiles:
                    fetch_x(*ORDER[i + 2])
                start_tile(t, m)
                for j in range(NCH):
                    hp = chunk_mms(t, m, j)
                    drain_chunk(t, m, j, hp, accs[(t, m)], gstates[(t, m)])
                finish_tile(t, m)
    nc.compile()
    return nc


def _prep_w(W_share, W_spcf, W_gate):
    # per-task column layout [gates 8 | shared e0..3 | specific e4..7], scaled
    wt = []
    for t in range(T):
        cols = np.concatenate(
            [
                W_gate[t],
                np.transpose(W_share, (1, 0, 2)).reshape(D, S * DE),
                np.transpose(W_spcf[t], (1, 0, 2)).reshape(D, P * DE),
            ],
            axis=1,
        ) * WS
        wt.append(cols)  # [D, TCOLS] f32

    big67 = np.concatenate(
        [wt[0][:, 512:2048], wt[0][:, 0:512], wt[0][:, 2048:], wt[1][:, 0:512], wt[1][:, 2048:]],
        axis=1,
    )
    wb67 = np.ascontiguousarray(
        big67.reshape(KT, 128, NB67)[2:8].transpose(1, 0, 2)
    ).astype(BF16)
    wb01 = np.ascontiguousarray(
        np.stack([w[:256].reshape(2, 128, TCOLS).transpose(1, 0, 2) for w in wt], 0)
    ).astype(BF16)
    big8 = np.concatenate(
        [wt[0][:256, 512:2048], wt[0][:256, 2048:], wt[1][:256, 2048:]], axis=1
    )
    w8 = np.ascontiguousarray(big8.reshape(2, 128, NB8).transpose(1, 0, 2)).astype(F8)
    wc0 = np.ascontiguousarray(
        np.stack(
            [w[:, 0:512].reshape(KT, 128, 512).transpose(1, 0, 2) for w in wt], 0
        )
    ).astype(BF16)
    return wb67, wb01, w8, wc0


def _prep_x(x, core, mt=MT):
    # [BC, D] -> [mt, 128p(d%128), KT, 128b] scaled; split k0-1 / k2-7
    xc = x[core * BC : core * BC + mt * 128] * XS
    xt = xc.reshape(mt, 128, KT, 128).transpose(0, 3, 2, 1)
    xb = np.ascontiguousarray(xt[:, :, 2:8]).astype(BF16)
    xb01 = np.ascontiguousarray(xt[:, :, 0:2]).astype(BF16)
    # sw-interleave for DoubleRowSwInterleave: [m, p, 256] with
    # col 2j = x_k0[:, 127-j], col 2j+1 = x_k1[:, 127-j]
    x8i = np.empty((mt, 128, 256), np.float32)
    x8i[:, :, 0::2] = xt[:, :, 0, ::-1]
    x8i[:, :, 1::2] = xt[:, :, 1, ::-1]
    x8 = np.ascontiguousarray(x8i).astype(F8)
    return xb, xb01, x8


def _prep_bias(b_share, b_spcf, b_gate):
    bec = np.zeros((T, TCOLS), np.float32)
    for t in range(T):
        bec[t, 0:NE] = np.asarray(b_gate, np.float32).reshape(T, NE)[t]
        bec[t, NE : NE + S * DE] = np.asarray(b_share, np.float32).reshape(S * DE)
        bec[t, NE + S * DE :] = np.asarray(b_spcf, np.float32).reshape(T, P * DE)[t]
    bec *= HS
    return np.ascontiguousarray(np.broadcast_to(bec[:, None], (T, 128, TCOLS)))


def kernel(x0, x1, W_share, b_share, W_spcf, b_spcf, W_gate, b_gate):
    from concourse.bass_utils import run_bass_kernel_spmd

    has_bias = bool(
        np.any(np.asarray(b_share)) or np.any(np.asarray(b_spcf)) or np.any(np.asarray(b_gate))
    )
    key = (MT, has_bias, USE_DR)
    if key not in _cache:
        _cache[key] = _build_bass(*key)
    nc = _cache[key]

    wb67, wb01, w8, wc0 = _prep_w(
        np.asarray(W_share, np.float32),
        np.asarray(W_spcf, np.float32),
        np.asarray(W_gate, np.float32),
    )
    xs = [np.asarray(x0, np.float32), np.asarray(x1, np.float32)]

    in_maps = []
    for c in range(NCORES):
        parts = [_prep_x(xs[t], c) for t in range(T)]
        m = {
            "xb": np.ascontiguousarray(np.stack([p[0] for p in parts], 0)),
            "xb01": np.ascontiguousarray(np.stack([p[1] for p in parts], 0)),
            "x8": np.ascontiguousarray(np.stack([p[2] for p in parts], 0)),
            "wb67": wb67,
            "wb01": wb01,
            "w8": w8,
            "wc0": wc0,
        }
        if has_bias:
            m["bec"] = _prep_bias(b_share, b_spcf, b_gate)
        in_maps.append(m)

    res = run_bass_kernel_spmd(nc, in_maps, core_ids=list(range(NCORES)))
    global _last_results
    _last_results = res

    outp = np.empty((B, T, DE), np.float32)
    for c in range(NCORES):
        oc = res.results[c]["out"]  # [T, MT, 128, DE]
        outp[c * BC : (c + 1) * BC] = oc.transpose(1, 2, 0, 3).reshape(BC, T, DE)
    return outp
